# revision 41
# baseline (speedup 1.0000x reference)
"""DeepSpeed-style fused residual+LayerNorm+MLP block on 8 trn2 NeuronCores.

Strategy: data-parallel over tokens (B*S = 16384 -> 2048 tokens/core).
Each core runs the full fused chain with replicated weights; no collectives.

v4: fp8e4 DoubleRow matmuls (2 fp8 weights/PE cell, ~2x over bf16 at N=512)
with the PE stream reduced to pure GEMM work:

  A1: h = x + r + bias; LayerNorm stats (bn_stats/bn_aggr)  [DVE]
      x/r tiles alternate between the sync and gpsimd DMA queues.
  A1n: rstd via ACT Sqrt (4 adjacent -> one table ctx) + DVE reciprocal;
      ln -> bf16 [DVE]
  A2: PE-transposes ln into feature-major bf16 (4 transposes of one k
      across the token tiles share one psum bank), then a single wide
      copy converts bf16 -> fp8e4 into per-k-pair lnT tiles.  The
      supertile-0 copies run on DVE (in-order with the LN chain); later
      supertiles' run on ScalarE inside the previous GEMM2's hc=1 phase.
      Dummy warmup matmuls on a scratch tile keep the PE HAM clock-gate
      warm through the DMA-bound startup.
  B:  interT[I,tok] = W1^T @ lnT (fp8 DoubleRow, fp32 PSUM; ps_g1
      triple-buffered against the GELU drain);
      exact-erf GELU with scale=1/128 + per-I bias on ScalarE -> fp8e4,
      written into per-ip-pair tiles so GEMM2 never waits the whole set.
  C:  out[tok,H] = interT^T @ W2 (W2 resident in SBUF as 8 chunk tiles,
      fp8 DoubleRow) in four (hc, tq-pair) quarters so the 4 ps_g2 banks
      rotate without stalls; epilogue: ACT Copy(scale=1/128), DVE adds
      h (+output_b).

Weights are scaled x128 host-side so |w|~2.5 sits in e4m3's normal range
(TRN e4m3 max +-240); the scale is undone for free in the GELU (ACT
computes func(in*scale+bias)) and by the epilogue Copy's scale.

Fine tile granularity matters: Tile tracks dependencies per tile, so
lnT / interT / W2 are split into pair/chunk tiles to keep readers from
conservatively waiting on a whole supertile's writes (this was worth
~16us at startup alone).

ACT FIFO per supertile: [32 GELU (gelu table)] -> [4 Sqrt (sqrt table)]
-> [epilogue + lnT Copies (Copy lives in every table set)] — two table
loads per supertile and no head-of-line block on the GELUs.

Startup is HBM-bound (~300 GB/s/core): the first-needed bytes
(x_0/r_0 split across queues, then W1 g0/g1 on sync, g2-g7 on gpsimd,
then supertile-1 tokens, then resident W2 chunks) are queued strictly
in deadline order. Final-supertile stores fan out over 3 queues.

Host-side prep (cheap, numpy): fold attn_nw into W1 rows, fold
attn_nb@W1+inter_b into a single GEMM1 bias, scale W1/W2 by 128 and
cast to fp8e4 (clipped to +-240).
"""

import numpy as np
import ml_dtypes

import concourse.bass as bass
import concourse.bacc as bacc
import concourse.mybir as mybir
import concourse.tile as tile
from concourse.tile import add_dep_helper
from concourse.bass_utils import run_bass_kernel_spmd

N_CORES = 8
B, S, H, I = 4, 4096, 1024, 4096
TOK = B * S              # 16384 tokens total
TPC = TOK // N_CORES     # 2048 tokens per core
P = 128
T_TILES = TPC // P       # 16 token tiles per core
ST = 4                   # token tiles per supertile
N_SUPER = T_TILES // ST  # 4 supertiles
ST_TOK = ST * P          # 512 tokens per supertile
KO1 = H // P             # 8 contraction subtiles for GEMM1
KP1 = KO1 // 2           # 4 k-pairs (DoubleRow)
IC = I // P              # 32 I-chunks
IP2 = IC // 2            # 16 ip-pairs (DoubleRow)
IG = 8                   # W1 i-groups (independent SBUF tiles)
ICG = IC // IG           # 4 I-chunks per group
HCW = 512                # output column chunk (1 PSUM bank of f32)
HC = H // HCW            # 2
W2B = 8                  # io-subtiles per resident W2 chunk tile
EPS = 1e-5
WSCALE = 128.0           # host-side fp8 weight scale (power of 2, exact)

_F32 = mybir.dt.float32
_BF16 = mybir.dt.bfloat16
_FP8 = mybir.dt.float8e4
_DR = mybir.MatmulPerfMode.DoubleRow

TRACE = False
LAST_RESULT = None


def _build_nc():
    nc = bacc.Bacc()
    x = nc.dram_tensor("x", (TPC, H), _F32, kind="ExternalInput")
    r = nc.dram_tensor("r", (TPC, H), _F32, kind="ExternalInput")
    w1 = nc.dram_tensor("w1", (H, I), _FP8, kind="ExternalInput")
    b1 = nc.dram_tensor("b1", (I,), _F32, kind="ExternalInput")
    w2 = nc.dram_tensor("w2", (I, H), _FP8, kind="ExternalInput")
    ab = nc.dram_tensor("ab", (H,), _F32, kind="ExternalInput")
    ob = nc.dram_tensor("ob", (H,), _F32, kind="ExternalInput")
    eye = nc.dram_tensor("eye", (P, P), _BF16, kind="ExternalInput")
    out = nc.dram_tensor("out", (TPC, H), _F32, kind="ExternalOutput")

    with tile.TileContext(nc) as tc:
        with (
            tc.tile_pool(name="consts", bufs=1) as consts,
            tc.tile_pool(name="w1p", bufs=1) as w1p,
            tc.tile_pool(name="w2p", bufs=1) as w2p,
            tc.tile_pool(name="hsup", bufs=2) as hsup,
            tc.tile_pool(name="xin", bufs=5) as xin,
            tc.tile_pool(name="rin", bufs=5) as rin,
            tc.tile_pool(name="lnp", bufs=4) as lnp,
            tc.tile_pool(name="lntp", bufs=8) as lntp,
            tc.tile_pool(name="intp", bufs=16) as intp,
            tc.tile_pool(name="resp", bufs=4) as resp,
            tc.tile_pool(name="stat", bufs=8) as stat,
            tc.tile_pool(name="ps_g1", bufs=3, space="PSUM") as ps_g1,
            tc.tile_pool(name="ps_g2", bufs=4, space="PSUM") as ps_g2,
            tc.tile_pool(name="ps_tr", bufs=1, space="PSUM") as ps_tr,
        ):
            eps_t = consts.tile([P, 1], _F32)
            nc.vector.memset(eps_t, EPS)

            ab_full = consts.tile([P, H], _F32)
            ab_ap = ab[:]
            nc.gpsimd.dma_start(
                out=ab_full,
                in_=bass.AP(tensor=ab_ap.tensor, offset=ab_ap.offset,
                            ap=[[0, P]] + list(ab_ap.ap)),
            )
            ident = consts.tile([P, P], _BF16)
            nc.gpsimd.dma_start(out=ident, in_=eye[:, :])
            scratch = consts.tile([P, HCW], _BF16)
            nc.vector.memset(scratch, 0.25)

            ob_full = consts.tile([P, H], _F32)
            b1_st = consts.tile([P, IC], _F32)
            nc.gpsimd.dma_start(out=b1_st, in_=b1[:].rearrange("(i p) -> p i", p=P))

            w1r = w1[:, :].rearrange("(ko p) i -> p ko i", p=P)
            w2r = w2[:, :].rearrange("(io p) h -> p io h", p=P)

            h_sups = [None] * N_SUPER
            a2_last = [None] * N_SUPER    # last lnT copy inst per supertile
            lnTs = [None] * N_SUPER       # fp8 per-k-pair tiles
            ln_ts = [None] * N_SUPER
            mvs = [None] * N_SUPER
            w1_ig = [None] * IG
            w2sb = [None] * (HC * 4)      # 8 resident chunk tiles

            x_tiles = {}
            r_tiles = {}

            def _last_inst():
                return nc.inst_map[next(reversed(nc.inst_map))]

            def emit_a1_dma(s):
                """x/r tile loads, alternating queues (2.1MB per queue)"""
                for t in range(ST):
                    g = s * ST + t
                    x_t = xin.tile([P, H], _F32, name=f"x_{g}", tag="x_t")
                    r_t = rin.tile([P, H], _F32, name=f"r_{g}", tag="r_t")
                    x_eng = nc.sync if t % 2 == 0 else nc.gpsimd
                    r_eng = nc.gpsimd if t % 2 == 0 else nc.sync
                    x_eng.dma_start(out=x_t, in_=x[g * P:(g + 1) * P, :])
                    r_eng.dma_start(out=r_t, in_=r[g * P:(g + 1) * P, :])
                    x_tiles[g] = x_t
                    r_tiles[g] = r_t

            def emit_a1_stats(s):
                """residual adds + LN stats (DVE only)"""
                h_sup = hsup.tile([P, ST, H], _F32, name=f"h_sup{s}", tag="h_sup")
                mv = stat.tile([P, ST, 2], _F32, name=f"mv_{s}", tag="mv")
                for t in range(ST):
                    g = s * ST + t
                    h_sl = h_sup[:, t, :]
                    nc.vector.tensor_add(h_sl, x_tiles[g], r_tiles[g])
                    nc.vector.tensor_add(h_sl, h_sl, ab_full)
                    stats = stat.tile([P, 2, 6], _F32, name=f"st_{g}", tag="stats")
                    for q in range(2):
                        nc.vector.bn_stats(out=stats[:, q, :],
                                           in_=h_sl[:, q * 512:(q + 1) * 512])
                    nc.vector.bn_aggr(out=mv[:, t, :], in_=stats)
                    if s == N_SUPER - 1:
                        # fold output bias into h here; its epilogue is the
                        # kernel tail, so halving the tail DVE work matters
                        nc.vector.tensor_add(h_sl, h_sl, ob_full)
                h_sups[s] = h_sup
                mvs[s] = mv

            def emit_a1_norm(s):
                """rstd (4 adjacent ACT Sqrt -> one table ctx) + bf16 ln"""
                mv = mvs[s]
                h_sup = h_sups[s]
                for t in range(ST):
                    nc.scalar.activation(
                        out=mv[:, t, 1:2], in_=mv[:, t, 1:2],
                        func=mybir.ActivationFunctionType.Sqrt,
                        bias=eps_t, scale=1.0)
                lns = []
                for t in range(ST):
                    nc.vector.reciprocal(out=mv[:, t, 1:2], in_=mv[:, t, 1:2])
                    g = s * ST + t
                    ln_t = lnp.tile([P, H], _BF16, name=f"ln_{g}", tag="ln_t")
                    nc.vector.tensor_scalar(
                        out=ln_t, in0=h_sup[:, t, :],
                        scalar1=mv[:, t, 0:1], scalar2=mv[:, t, 1:2],
                        op0=mybir.AluOpType.subtract, op1=mybir.AluOpType.mult,
                    )
                    lns.append(ln_t)
                ln_ts[s] = lns
                lnTs[s] = [lntp.tile([P, 2, ST_TOK], _FP8, name=f"lnT{s}_{j}",
                                     tag="lnT") for j in range(KP1)]

            def emit_a2_batch(s, k, on_dve=False):
                """4 PE transposes (one k across all 4 token tiles) into one
                psum bank + a single wide copy -> fp8 lnT pair half"""
                trp = ps_tr.tile([P, ST * P], _BF16, name=f"tr_{s}_{k}",
                                 tag="trp")
                for t in range(ST):
                    nc.tensor.transpose(trp[:, t * P:(t + 1) * P],
                                        ln_ts[s][t][:, k * P:(k + 1) * P],
                                        ident)
                dst = lnTs[s][k // 2][:, k % 2, :]
                nc.scalar.copy(out=dst, in_=trp)
                a2_last[s] = _last_inst()

            def emit_b(s):
                """GEMM1 (fp8 DoubleRow) + bias + exact GELU -> interT fp8"""
                pairs = [intp.tile([P, 2, ST_TOK], _FP8, name=f"int{s}_{j}",
                                   tag="interT") for j in range(IP2)]
                for i in range(IC):
                    pg1 = ps_g1.tile([P, ST_TOK], _F32, name=f"pg1_{s}_{i}",
                                     tag="pg1")
                    for kj in range(KP1):
                        nc.tensor.matmul(
                            pg1,
                            w1_ig[i // ICG][:, 2 * kj:2 * kj + 2,
                                            (i % ICG) * P:(i % ICG + 1) * P],
                            lnTs[s][kj][:, :, :],
                            start=(kj == 0), stop=(kj == KP1 - 1),
                            perf_mode=_DR)
                    nc.scalar.activation(out=pairs[i // 2][:, i % 2, :],
                                         in_=pg1,
                                         func=mybir.ActivationFunctionType.Gelu,
                                         bias=b1_st[:, i:i + 1],
                                         scale=1.0 / WSCALE)
                return pairs

            def emit_c(s, interT, next_s):
                """GEMM2 (fp8 DoubleRow, resident W2 chunks) in 4
                (hc, tq-pair) quarters + epilogue. s+1's lnT fp8 converts
                ride in the hc=1 quarters."""
                last = s == N_SUPER - 1
                a2_j = 0
                for hc in range(HC):
                    for th in range(2):
                        tqs = (2 * th, 2 * th + 1)
                        pg2s = [ps_g2.tile([P, HCW], _F32,
                                           name=f"pg2_{s}_{hc}_{tq}", tag="pg2")
                                for tq in tqs]
                        for ip in range(0, IC, 2):
                            wt = w2sb[hc * 4 + ip // W2B]
                            io = ip % W2B
                            for j, tq in enumerate(tqs):
                                nc.tensor.matmul(
                                    pg2s[j],
                                    interT[ip // 2][:, :, tq * P:(tq + 1) * P],
                                    wt[:, io:io + 2, :],
                                    start=(ip == 0), stop=(ip == IC - 2),
                                    perf_mode=_DR)
                        if hc == 1 and next_s is not None:
                            for _ in range(4):
                                if a2_j < KO1:
                                    emit_a2_batch(next_s, a2_j)
                                    a2_j += 1
                        for j, tq in enumerate(tqs):
                            g = s * ST + tq
                            res_h = resp.tile([P, HCW], _F32,
                                              name=f"res_{s}_{hc}_{tq}",
                                              tag="res_h")
                            nc.scalar.activation(
                                out=res_h, in_=pg2s[j],
                                func=mybir.ActivationFunctionType.Copy,
                                scale=1.0 / WSCALE)
                            nc.vector.tensor_add(
                                res_h, res_h,
                                h_sups[s][:, tq, hc * HCW:(hc + 1) * HCW])
                            if not last:
                                nc.vector.tensor_add(
                                    res_h, res_h,
                                    ob_full[:, hc * HCW:(hc + 1) * HCW])
                            if last:
                                # kernel tail: fan the stores over 3 queues
                                st_eng = (nc.sync, nc.gpsimd,
                                          nc.scalar)[(2 * hc + th + j) % 3]
                            else:
                                st_eng = nc.gpsimd
                            st_eng.dma_start(
                                out=out[g * P:(g + 1) * P,
                                        hc * HCW:(hc + 1) * HCW],
                                in_=res_h)

            # ---- emission schedule (deadline-ordered DMA queues) ----
            emit_a1_dma(0)                  # x/r supertile 0, both queues
            # W1 groups by deadline: g0,g1 on sync and g2,g3 on gpsimd right
            # behind the supertile-0 tokens; g4..g7 after supertile-1's
            # tokens (those feed the sqrt/ln chain the ACT stream blocks on)
            kh = KO1 // 2

            def emit_w1_group(ig, eng):
                w1t = w1p.tile([P, KO1, ICG * P], _FP8, name=f"w1_{ig}",
                               tag=f"w1_{ig}")
                for q in range(2):
                    eng.dma_start(
                        out=w1t[:, q * kh:(q + 1) * kh, :],
                        in_=w1r[:, q * kh:(q + 1) * kh,
                                ig * ICG * P:(ig + 1) * ICG * P])
                w1_ig[ig] = w1t

            # g0,g1,g4,g5 on sync; g2,g3 on gpsimd -- this leaves only 1MB
            # of W1 ahead of supertile-1's gpsimd token half, so the LN
            # stats chain (which the ACT stream blocks on) unblocks ~25us
            # earlier; g6,g7 follow tokens-1 on gpsimd (needed ~20us into
            # GEMM1, landing well before)
            for ig in (0, 1, 4, 5):
                emit_w1_group(ig, nc.sync)
            for ig in (2, 3):
                emit_w1_group(ig, nc.gpsimd)
            ob_ap = ob[:]
            nc.gpsimd.dma_start(
                out=ob_full,
                in_=bass.AP(tensor=ob_ap.tensor, offset=ob_ap.offset,
                            ap=[[0, P]] + list(ob_ap.ap)),
            )
            emit_a1_stats(0)
            emit_a1_norm(0)
            for k in range(KO1):            # supertile 0 fp8 lnT up front
                emit_a2_batch(0, k)
            for s in range(N_SUPER):
                if s + 1 < N_SUPER:
                    emit_a1_dma(s + 1)
                if s == 0:
                    for ig in (6, 7):       # w1 tail behind tokens-1
                        emit_w1_group(ig, nc.gpsimd)
                    # resident W2 chunks on sync behind supertile-1 tokens;
                    # hc=0 chunks first (needed from GEMM2(0) q1)
                    for hcq in range(HC):
                        for jb in range(4):
                            wt = w2p.tile([P, W2B, HCW], _FP8,
                                          name=f"w2_{hcq}_{jb}",
                                          tag=f"w2_{hcq}_{jb}")
                            nc.sync.dma_start(
                                out=wt,
                                in_=w2r[:, W2B * jb:W2B * (jb + 1),
                                        hcq * HCW:(hcq + 1) * HCW])
                            w2sb[hcq * 4 + jb] = wt
                interT = emit_b(s)
                if s + 1 < N_SUPER:
                    # ACT: sqrts land after the GELUs; DVE: A1 chain + ln;
                    # sync ring: the 4 XBAR transposes
                    emit_a1_stats(s + 1)
                    emit_a1_norm(s + 1)
                emit_c(s, interT, s + 1 if s + 1 < N_SUPER else None)

    nc.finalize()
    return nc


def kernel(input, residual, bias, attn_nw, attn_nb, inter_w, inter_b,
           output_w, output_b):
    global LAST_RESULT
    input = np.asarray(input, dtype=np.float32)
    residual = np.asarray(residual, dtype=np.float32)
    bias = np.asarray(bias, dtype=np.float32)
    attn_nw = np.asarray(attn_nw, dtype=np.float32)
    attn_nb = np.asarray(attn_nb, dtype=np.float32)
    inter_w = np.asarray(inter_w, dtype=np.float32)
    inter_b = np.asarray(inter_b, dtype=np.float32)
    output_w = np.asarray(output_w, dtype=np.float32)
    output_b = np.asarray(output_b, dtype=np.float32)

    x = np.ascontiguousarray(input.reshape(TOK, H))
    r = np.ascontiguousarray(residual.reshape(TOK, H))
    # fold LN affine params into GEMM1 weight/bias (exact algebra):
    #   (std*nw + nb) @ W1 + b1 == std @ (nw[:,None]*W1) + (nb @ W1 + b1)
    # then scale weights x128 into e4m3's normal range (TRN max +-240);
    # the GELU's scale=1/128 and the epilogue Copy scale undo it exactly.
    w1p = np.clip((attn_nw[:, None] * inter_w) * WSCALE, -240.0, 240.0)
    w1p = np.ascontiguousarray(w1p).astype(ml_dtypes.float8_e4m3)
    b1p = (attn_nb @ inter_w + inter_b).astype(np.float32)
    w2p = np.clip(output_w * WSCALE, -240.0, 240.0)
    w2p = np.ascontiguousarray(w2p).astype(ml_dtypes.float8_e4m3)
    eye = np.eye(P, dtype=ml_dtypes.bfloat16)

    nc = _build_nc()
    in_maps = []
    for c in range(N_CORES):
        in_maps.append({
            "x": np.ascontiguousarray(x[c * TPC:(c + 1) * TPC]),
            "r": np.ascontiguousarray(r[c * TPC:(c + 1) * TPC]),
            "w1": w1p, "b1": b1p, "w2": w2p,
            "ab": bias, "ob": output_b, "eye": eye,
        })
    res = run_bass_kernel_spmd(nc, in_maps, core_ids=list(range(N_CORES)),
                               trace=TRACE)
    LAST_RESULT = res
    out = np.concatenate([res.results[c]["out"] for c in range(N_CORES)], axis=0)
    return np.ascontiguousarray(out.reshape(B, S, H)).astype(np.float32)


# revision 52
# speedup vs baseline: 1.0095x; 1.0095x over previous
"""DeepSpeed-style fused residual+LayerNorm+MLP block on 8 trn2 NeuronCores.

Strategy: data-parallel over tokens (B*S = 16384 -> 2048 tokens/core).
Each core runs the full fused chain with replicated weights; no collectives.

v4: fp8e4 DoubleRow matmuls (2 fp8 weights/PE cell, ~2x over bf16 at N=512)
with the PE stream reduced to pure GEMM work:

  A1: h = x + r + bias; LayerNorm stats (bn_stats/bn_aggr)  [DVE]
      x/r tiles alternate between the sync and gpsimd DMA queues.
  A1n: rstd via ACT Sqrt (4 adjacent -> one table ctx) + DVE reciprocal;
      ln -> bf16 [DVE]
  A2: PE-transposes ln into feature-major bf16 (4 transposes of one k
      across the token tiles share one psum bank), then a single wide
      copy converts bf16 -> fp8e4 into per-k-pair lnT tiles.  The
      supertile-0 copies run on DVE (in-order with the LN chain); later
      supertiles' run on ScalarE inside the previous GEMM2's hc=1 phase.
      Dummy warmup matmuls on a scratch tile keep the PE HAM clock-gate
      warm through the DMA-bound startup.
  B:  interT[I,tok] = W1^T @ lnT (fp8 DoubleRow, fp32 PSUM; ps_g1
      triple-buffered against the GELU drain);
      exact-erf GELU with scale=1/128 + per-I bias on ScalarE -> fp8e4,
      written into per-ip-pair tiles so GEMM2 never waits the whole set.
  C:  out[tok,H] = interT^T @ W2 (W2 resident in SBUF as 8 chunk tiles,
      fp8 DoubleRow) in four (hc, tq-pair) quarters so the 4 ps_g2 banks
      rotate without stalls; epilogue: ACT Copy(scale=1/128), DVE adds
      h (+output_b).

Weights are scaled x128 host-side so |w|~2.5 sits in e4m3's normal range
(TRN e4m3 max +-240); the scale is undone for free in the GELU (ACT
computes func(in*scale+bias)) and by the epilogue Copy's scale.

Fine tile granularity matters: Tile tracks dependencies per tile, so
lnT / interT / W2 are split into pair/chunk tiles to keep readers from
conservatively waiting on a whole supertile's writes (this was worth
~16us at startup alone).

ACT FIFO per supertile: [32 GELU (gelu table)] -> [4 Sqrt (sqrt table)]
-> [epilogue + lnT Copies (Copy lives in every table set)] — two table
loads per supertile and no head-of-line block on the GELUs.

Startup is HBM-bound (~300 GB/s/core): the first-needed bytes
(x_0/r_0 split across queues, then W1 g0/g1 on sync, g2-g7 on gpsimd,
then supertile-1 tokens, then resident W2 chunks) are queued strictly
in deadline order. Final-supertile stores fan out over 3 queues.

Host-side prep (cheap, numpy): fold attn_nw into W1 rows, fold
attn_nb@W1+inter_b into a single GEMM1 bias, scale W1/W2 by 128 and
cast to fp8e4 (clipped to +-240).
"""

import numpy as np
import ml_dtypes

import concourse.bass as bass
import concourse.bacc as bacc
import concourse.mybir as mybir
import concourse.tile as tile
from concourse.tile import add_dep_helper
from concourse.bass_utils import run_bass_kernel_spmd

N_CORES = 8
B, S, H, I = 4, 4096, 1024, 4096
TOK = B * S              # 16384 tokens total
TPC = TOK // N_CORES     # 2048 tokens per core
P = 128
T_TILES = TPC // P       # 16 token tiles per core
ST = 4                   # token tiles per supertile
N_SUPER = T_TILES // ST  # 4 supertiles
ST_TOK = ST * P          # 512 tokens per supertile
KO1 = H // P             # 8 contraction subtiles for GEMM1
KP1 = KO1 // 2           # 4 k-pairs (DoubleRow)
IC = I // P              # 32 I-chunks
IP2 = IC // 2            # 16 ip-pairs (DoubleRow)
IG = 8                   # W1 i-groups (independent SBUF tiles)
ICG = IC // IG           # 4 I-chunks per group
HCW = 512                # output column chunk (1 PSUM bank of f32)
HC = H // HCW            # 2
W2B = 8                  # io-subtiles per resident W2 chunk tile
EPS = 1e-5
WSCALE = 128.0           # host-side fp8 weight scale (power of 2, exact)

_F32 = mybir.dt.float32
_BF16 = mybir.dt.bfloat16
_FP8 = mybir.dt.float8e4
_DR = mybir.MatmulPerfMode.DoubleRow

TRACE = False
LAST_RESULT = None


def _build_nc():
    nc = bacc.Bacc()
    x = nc.dram_tensor("x", (TPC, H), _F32, kind="ExternalInput")
    r = nc.dram_tensor("r", (TPC, H), _F32, kind="ExternalInput")
    w1 = nc.dram_tensor("w1", (H, I), _FP8, kind="ExternalInput")
    b1 = nc.dram_tensor("b1", (I,), _F32, kind="ExternalInput")
    w2 = nc.dram_tensor("w2", (I, H), _FP8, kind="ExternalInput")
    ab = nc.dram_tensor("ab", (H,), _BF16, kind="ExternalInput")
    ob = nc.dram_tensor("ob", (H,), _BF16, kind="ExternalInput")
    eye = nc.dram_tensor("eye", (P, P), _BF16, kind="ExternalInput")
    out = nc.dram_tensor("out", (TPC, H), _F32, kind="ExternalOutput")

    with tile.TileContext(nc) as tc:
        with (
            tc.tile_pool(name="consts", bufs=1) as consts,
            tc.tile_pool(name="w1p", bufs=1) as w1p,
            tc.tile_pool(name="w2p", bufs=1) as w2p,
            tc.tile_pool(name="hsup", bufs=2) as hsup,
            tc.tile_pool(name="xin", bufs=5) as xin,
            tc.tile_pool(name="rin", bufs=5) as rin,
            tc.tile_pool(name="lnp", bufs=4) as lnp,
            tc.tile_pool(name="lntp", bufs=8) as lntp,
            tc.tile_pool(name="intp", bufs=16) as intp,
            tc.tile_pool(name="resp", bufs=4) as resp,
            tc.tile_pool(name="stat", bufs=8) as stat,
            tc.tile_pool(name="ps_g1", bufs=3, space="PSUM") as ps_g1,
            tc.tile_pool(name="ps_g2", bufs=4, space="PSUM") as ps_g2,
            tc.tile_pool(name="ps_tr", bufs=1, space="PSUM") as ps_tr,
        ):
            eps_t = consts.tile([P, 1], _F32)
            nc.vector.memset(eps_t, EPS)

            # biases are ~0.02-std, so they ship as bf16: halves the 512KB
            # partition-broadcast DMAs (ab sits ahead of r0_t0 on gpsimd)
            ab_full = consts.tile([P, H], _BF16)
            ab_ap = ab[:]
            nc.gpsimd.dma_start(
                out=ab_full,
                in_=bass.AP(tensor=ab_ap.tensor, offset=ab_ap.offset,
                            ap=[[0, P]] + list(ab_ap.ap)),
            )
            ident = consts.tile([P, P], _BF16)
            nc.gpsimd.dma_start(out=ident, in_=eye[:, :])
            scratch = consts.tile([P, HCW], _BF16)
            nc.vector.memset(scratch, 0.25)

            ob_full = consts.tile([P, H], _BF16)
            b1_st = consts.tile([P, IC], _F32)
            nc.gpsimd.dma_start(out=b1_st, in_=b1[:].rearrange("(i p) -> p i", p=P))

            w1r = w1[:, :].rearrange("(ko p) i -> p ko i", p=P)
            w2r = w2[:, :].rearrange("(io p) h -> p io h", p=P)

            h_sups = [None] * N_SUPER
            a2_last = [None] * N_SUPER    # last lnT copy inst per supertile
            lnTs = [None] * N_SUPER       # fp8 per-k-pair tiles
            ln_ts = [None] * N_SUPER
            mvs = [None] * N_SUPER
            w1_ig = [None] * IG
            w2sb = [None] * (HC * 4)      # 8 resident chunk tiles

            x_tiles = {}
            r_tiles = {}

            def _last_inst():
                return nc.inst_map[next(reversed(nc.inst_map))]

            def emit_a1_dma(s):
                """x/r tile loads, alternating queues (2.1MB per queue)"""
                for t in range(ST):
                    g = s * ST + t
                    x_t = xin.tile([P, H], _F32, name=f"x_{g}", tag="x_t")
                    r_t = rin.tile([P, H], _F32, name=f"r_{g}", tag="r_t")
                    x_eng = nc.sync if t % 2 == 0 else nc.gpsimd
                    r_eng = nc.gpsimd if t % 2 == 0 else nc.sync
                    x_eng.dma_start(out=x_t, in_=x[g * P:(g + 1) * P, :])
                    r_eng.dma_start(out=r_t, in_=r[g * P:(g + 1) * P, :])
                    x_tiles[g] = x_t
                    r_tiles[g] = r_t

            def emit_a1_stats(s):
                """residual adds + LN stats (DVE only)"""
                h_sup = hsup.tile([P, ST, H], _F32, name=f"h_sup{s}", tag="h_sup")
                mv = stat.tile([P, ST, 2], _F32, name=f"mv_{s}", tag="mv")
                for t in range(ST):
                    g = s * ST + t
                    h_sl = h_sup[:, t, :]
                    nc.vector.tensor_add(h_sl, x_tiles[g], r_tiles[g])
                    nc.vector.tensor_add(h_sl, h_sl, ab_full)
                    stats = stat.tile([P, 2, 6], _F32, name=f"st_{g}", tag="stats")
                    for q in range(2):
                        nc.vector.bn_stats(out=stats[:, q, :],
                                           in_=h_sl[:, q * 512:(q + 1) * 512])
                    nc.vector.bn_aggr(out=mv[:, t, :], in_=stats)
                    if s == N_SUPER - 1:
                        # fold output bias into h here; its epilogue is the
                        # kernel tail, so halving the tail DVE work matters
                        nc.vector.tensor_add(h_sl, h_sl, ob_full)
                h_sups[s] = h_sup
                mvs[s] = mv

            def emit_a1_norm(s):
                """rstd (4 adjacent ACT Sqrt -> one table ctx) + bf16 ln"""
                mv = mvs[s]
                h_sup = h_sups[s]
                for t in range(ST):
                    nc.scalar.activation(
                        out=mv[:, t, 1:2], in_=mv[:, t, 1:2],
                        func=mybir.ActivationFunctionType.Sqrt,
                        bias=eps_t, scale=1.0)
                lns = []
                for t in range(ST):
                    nc.vector.reciprocal(out=mv[:, t, 1:2], in_=mv[:, t, 1:2])
                    g = s * ST + t
                    ln_t = lnp.tile([P, H], _BF16, name=f"ln_{g}", tag="ln_t")
                    nc.vector.tensor_scalar(
                        out=ln_t, in0=h_sup[:, t, :],
                        scalar1=mv[:, t, 0:1], scalar2=mv[:, t, 1:2],
                        op0=mybir.AluOpType.subtract, op1=mybir.AluOpType.mult,
                    )
                    lns.append(ln_t)
                ln_ts[s] = lns
                lnTs[s] = [lntp.tile([P, 2, ST_TOK], _FP8, name=f"lnT{s}_{j}",
                                     tag="lnT") for j in range(KP1)]

            def emit_a2_batch(s, k, on_dve=False):
                """4 PE transposes (one k across all 4 token tiles) into one
                psum bank + a single wide copy -> fp8 lnT pair half"""
                trp = ps_tr.tile([P, ST * P], _BF16, name=f"tr_{s}_{k}",
                                 tag="trp")
                for t in range(ST):
                    nc.tensor.transpose(trp[:, t * P:(t + 1) * P],
                                        ln_ts[s][t][:, k * P:(k + 1) * P],
                                        ident)
                dst = lnTs[s][k // 2][:, k % 2, :]
                nc.scalar.copy(out=dst, in_=trp)
                a2_last[s] = _last_inst()

            def emit_b(s):
                """GEMM1 (fp8 DoubleRow) + bias + exact GELU -> interT fp8"""
                pairs = [intp.tile([P, 2, ST_TOK], _FP8, name=f"int{s}_{j}",
                                   tag="interT") for j in range(IP2)]
                for i in range(IC):
                    pg1 = ps_g1.tile([P, ST_TOK], _F32, name=f"pg1_{s}_{i}",
                                     tag="pg1")
                    for kj in range(KP1):
                        nc.tensor.matmul(
                            pg1,
                            w1_ig[i // ICG][:, 2 * kj:2 * kj + 2,
                                            (i % ICG) * P:(i % ICG + 1) * P],
                            lnTs[s][kj][:, :, :],
                            start=(kj == 0), stop=(kj == KP1 - 1),
                            perf_mode=_DR)
                    nc.scalar.activation(out=pairs[i // 2][:, i % 2, :],
                                         in_=pg1,
                                         func=mybir.ActivationFunctionType.Gelu,
                                         bias=b1_st[:, i:i + 1],
                                         scale=1.0 / WSCALE)
                return pairs

            def emit_c(s, interT, next_s):
                """GEMM2 (fp8 DoubleRow, resident W2 chunks) in 4
                (hc, tq-pair) quarters + epilogue. s+1's lnT fp8 converts
                ride in the hc=1 quarters."""
                last = s == N_SUPER - 1
                a2_j = 0
                for hc in range(HC):
                    for th in range(2):
                        tqs = (2 * th, 2 * th + 1)
                        pg2s = [ps_g2.tile([P, HCW], _F32,
                                           name=f"pg2_{s}_{hc}_{tq}", tag="pg2")
                                for tq in tqs]
                        for ipi, ip in enumerate(range(0, IC, 2)):
                            wt = w2sb[hc * 4 + ip // W2B]
                            io = ip % W2B
                            for j, tq in enumerate(tqs):
                                nc.tensor.matmul(
                                    pg2s[j],
                                    interT[ip // 2][:, :, tq * P:(tq + 1) * P],
                                    wt[:, io:io + 2, :],
                                    start=(ip == 0), stop=(ip == IC - 2),
                                    perf_mode=_DR)
                            if (hc == 1 and next_s is not None
                                    and ipi % 4 == 3 and a2_j < KO1):
                                emit_a2_batch(next_s, a2_j)
                                a2_j += 1
                        for j, tq in enumerate(tqs):
                            g = s * ST + tq
                            res_h = resp.tile([P, HCW], _F32,
                                              name=f"res_{s}_{hc}_{tq}",
                                              tag="res_h")
                            nc.scalar.activation(
                                out=res_h, in_=pg2s[j],
                                func=mybir.ActivationFunctionType.Copy,
                                scale=1.0 / WSCALE)
                            nc.vector.tensor_add(
                                res_h, res_h,
                                h_sups[s][:, tq, hc * HCW:(hc + 1) * HCW])
                            if not last:
                                nc.vector.tensor_add(
                                    res_h, res_h,
                                    ob_full[:, hc * HCW:(hc + 1) * HCW])
                            if last:
                                # kernel tail: fan the stores over 3 queues
                                st_eng = (nc.sync, nc.gpsimd,
                                          nc.scalar)[(2 * hc + th + j) % 3]
                            else:
                                st_eng = nc.gpsimd
                            st_eng.dma_start(
                                out=out[g * P:(g + 1) * P,
                                        hc * HCW:(hc + 1) * HCW],
                                in_=res_h)

            # ---- emission schedule (deadline-ordered DMA queues) ----
            emit_a1_dma(0)                  # x/r supertile 0, both queues
            # W1 groups by deadline: g0,g1 on sync and g2,g3 on gpsimd right
            # behind the supertile-0 tokens; g4..g7 after supertile-1's
            # tokens (those feed the sqrt/ln chain the ACT stream blocks on)
            kh = KO1 // 2

            def emit_w1_group(ig, eng):
                w1t = w1p.tile([P, KO1, ICG * P], _FP8, name=f"w1_{ig}",
                               tag=f"w1_{ig}")
                for q in range(2):
                    eng.dma_start(
                        out=w1t[:, q * kh:(q + 1) * kh, :],
                        in_=w1r[:, q * kh:(q + 1) * kh,
                                ig * ICG * P:(ig + 1) * ICG * P])
                w1_ig[ig] = w1t

            for ig in range(IG):
                emit_w1_group(ig, nc.sync if ig < 2 else nc.gpsimd)
            ob_ap = ob[:]
            nc.gpsimd.dma_start(
                out=ob_full,
                in_=bass.AP(tensor=ob_ap.tensor, offset=ob_ap.offset,
                            ap=[[0, P]] + list(ob_ap.ap)),
            )
            emit_a1_stats(0)
            emit_a1_norm(0)
            for k in range(KO1):            # supertile 0 fp8 lnT up front
                emit_a2_batch(0, k)
            for s in range(N_SUPER):
                if s + 1 < N_SUPER:
                    emit_a1_dma(s + 1)
                if s == 0:
                    # resident W2 chunks on sync behind supertile-1 tokens;
                    # hc=0 chunks first (needed from GEMM2(0) q1)
                    for hcq in range(HC):
                        for jb in range(4):
                            wt = w2p.tile([P, W2B, HCW], _FP8,
                                          name=f"w2_{hcq}_{jb}",
                                          tag=f"w2_{hcq}_{jb}")
                            nc.sync.dma_start(
                                out=wt,
                                in_=w2r[:, W2B * jb:W2B * (jb + 1),
                                        hcq * HCW:(hcq + 1) * HCW])
                            w2sb[hcq * 4 + jb] = wt
                interT = emit_b(s)
                if s + 1 < N_SUPER:
                    # ACT: sqrts land after the GELUs; DVE: A1 chain + ln;
                    # sync ring: the 4 XBAR transposes
                    emit_a1_stats(s + 1)
                    emit_a1_norm(s + 1)
                emit_c(s, interT, s + 1 if s + 1 < N_SUPER else None)

    nc.finalize()
    return nc


def kernel(input, residual, bias, attn_nw, attn_nb, inter_w, inter_b,
           output_w, output_b):
    global LAST_RESULT
    input = np.asarray(input, dtype=np.float32)
    residual = np.asarray(residual, dtype=np.float32)
    bias = np.asarray(bias, dtype=np.float32)
    attn_nw = np.asarray(attn_nw, dtype=np.float32)
    attn_nb = np.asarray(attn_nb, dtype=np.float32)
    inter_w = np.asarray(inter_w, dtype=np.float32)
    inter_b = np.asarray(inter_b, dtype=np.float32)
    output_w = np.asarray(output_w, dtype=np.float32)
    output_b = np.asarray(output_b, dtype=np.float32)

    x = np.ascontiguousarray(input.reshape(TOK, H))
    r = np.ascontiguousarray(residual.reshape(TOK, H))
    # fold LN affine params into GEMM1 weight/bias (exact algebra):
    #   (std*nw + nb) @ W1 + b1 == std @ (nw[:,None]*W1) + (nb @ W1 + b1)
    # then scale weights x128 into e4m3's normal range (TRN max +-240);
    # the GELU's scale=1/128 and the epilogue Copy scale undo it exactly.
    w1p = np.clip((attn_nw[:, None] * inter_w) * WSCALE, -240.0, 240.0)
    w1p = np.ascontiguousarray(w1p).astype(ml_dtypes.float8_e4m3)
    b1p = (attn_nb @ inter_w + inter_b).astype(np.float32)
    w2p = np.clip(output_w * WSCALE, -240.0, 240.0)
    w2p = np.ascontiguousarray(w2p).astype(ml_dtypes.float8_e4m3)
    eye = np.eye(P, dtype=ml_dtypes.bfloat16)

    nc = _build_nc()
    in_maps = []
    for c in range(N_CORES):
        in_maps.append({
            "x": np.ascontiguousarray(x[c * TPC:(c + 1) * TPC]),
            "r": np.ascontiguousarray(r[c * TPC:(c + 1) * TPC]),
            "w1": w1p, "b1": b1p, "w2": w2p,
            "ab": bias.astype(ml_dtypes.bfloat16),
            "ob": output_b.astype(ml_dtypes.bfloat16), "eye": eye,
        })
    res = run_bass_kernel_spmd(nc, in_maps, core_ids=list(range(N_CORES)),
                               trace=TRACE)
    LAST_RESULT = res
    out = np.concatenate([res.results[c]["out"] for c in range(N_CORES)], axis=0)
    return np.ascontiguousarray(out.reshape(B, S, H)).astype(np.float32)


# revision 54
# speedup vs baseline: 1.0240x; 1.0144x over previous
"""DeepSpeed-style fused residual+LayerNorm+MLP block on 8 trn2 NeuronCores.

Strategy: data-parallel over tokens (B*S = 16384 -> 2048 tokens/core).
Each core runs the full fused chain with replicated weights; no collectives.

v4: fp8e4 DoubleRow matmuls (2 fp8 weights/PE cell, ~2x over bf16 at N=512)
with the PE stream reduced to pure GEMM work:

  A1: h = x + r + bias; LayerNorm stats (bn_stats/bn_aggr)  [DVE]
      x/r tiles alternate between the sync and gpsimd DMA queues.
  A1n: rstd via ACT Sqrt (4 adjacent -> one table ctx) + DVE reciprocal;
      ln -> bf16 [DVE]
  A2: PE-transposes ln into feature-major bf16 (4 transposes of one k
      across the token tiles share one psum bank), then a single wide
      copy converts bf16 -> fp8e4 into per-k-pair lnT tiles.  The
      supertile-0 copies run on DVE (in-order with the LN chain); later
      supertiles' run on ScalarE inside the previous GEMM2's hc=1 phase.
      Dummy warmup matmuls on a scratch tile keep the PE HAM clock-gate
      warm through the DMA-bound startup.
  B:  interT[I,tok] = W1^T @ lnT (fp8 DoubleRow, fp32 PSUM; ps_g1
      triple-buffered against the GELU drain);
      exact-erf GELU with scale=1/128 + per-I bias on ScalarE -> fp8e4,
      written into per-ip-pair tiles so GEMM2 never waits the whole set.
  C:  out[tok,H] = interT^T @ W2 (W2 resident in SBUF as 8 chunk tiles,
      fp8 DoubleRow) in four (hc, tq-pair) quarters so the 4 ps_g2 banks
      rotate without stalls; epilogue: ACT Copy(scale=1/128), DVE adds
      h (+output_b).

Weights are scaled x128 host-side so |w|~2.5 sits in e4m3's normal range
(TRN e4m3 max +-240); the scale is undone for free in the GELU (ACT
computes func(in*scale+bias)) and by the epilogue Copy's scale.

Fine tile granularity matters: Tile tracks dependencies per tile, so
lnT / interT / W2 are split into pair/chunk tiles to keep readers from
conservatively waiting on a whole supertile's writes (this was worth
~16us at startup alone).

ACT FIFO per supertile: [32 GELU (gelu table)] -> [4 Sqrt (sqrt table)]
-> [epilogue + lnT Copies (Copy lives in every table set)] — two table
loads per supertile and no head-of-line block on the GELUs.

Startup is HBM-bound (~300 GB/s/core): the first-needed bytes
(x_0/r_0 split across queues, then W1 g0/g1 on sync, g2-g7 on gpsimd,
then supertile-1 tokens, then resident W2 chunks) are queued strictly
in deadline order. Final-supertile stores fan out over 3 queues.

Host-side prep (cheap, numpy): fold attn_nw into W1 rows, fold
attn_nb@W1+inter_b into a single GEMM1 bias, scale W1/W2 by 128 and
cast to fp8e4 (clipped to +-240).
"""

import numpy as np
import ml_dtypes

import concourse.bass as bass
import concourse.bacc as bacc
import concourse.mybir as mybir
import concourse.tile as tile
from concourse.tile import add_dep_helper
from concourse.bass_utils import run_bass_kernel_spmd

N_CORES = 8
B, S, H, I = 4, 4096, 1024, 4096
TOK = B * S              # 16384 tokens total
TPC = TOK // N_CORES     # 2048 tokens per core
P = 128
T_TILES = TPC // P       # 16 token tiles per core
ST = 4                   # token tiles per supertile
N_SUPER = T_TILES // ST  # 4 supertiles
ST_TOK = ST * P          # 512 tokens per supertile
KO1 = H // P             # 8 contraction subtiles for GEMM1
KP1 = KO1 // 2           # 4 k-pairs (DoubleRow)
IC = I // P              # 32 I-chunks
IP2 = IC // 2            # 16 ip-pairs (DoubleRow)
IG = 8                   # W1 i-groups (independent SBUF tiles)
ICG = IC // IG           # 4 I-chunks per group
HCW = 512                # output column chunk (1 PSUM bank of f32)
HC = H // HCW            # 2
W2B = 8                  # io-subtiles per resident W2 chunk tile
EPS = 1e-5
WSCALE = 128.0           # host-side fp8 weight scale (power of 2, exact)

_F32 = mybir.dt.float32
_BF16 = mybir.dt.bfloat16
_FP8 = mybir.dt.float8e4
_DR = mybir.MatmulPerfMode.DoubleRow

TRACE = False
LAST_RESULT = None


def _build_nc():
    nc = bacc.Bacc()
    x = nc.dram_tensor("x", (TPC, H), _F32, kind="ExternalInput")
    r = nc.dram_tensor("r", (TPC, H), _F32, kind="ExternalInput")
    w1 = nc.dram_tensor("w1", (H, I), _FP8, kind="ExternalInput")
    b1 = nc.dram_tensor("b1", (I,), _F32, kind="ExternalInput")
    w2 = nc.dram_tensor("w2", (I, H), _FP8, kind="ExternalInput")
    ab = nc.dram_tensor("ab", (H,), _BF16, kind="ExternalInput")
    ob = nc.dram_tensor("ob", (H,), _BF16, kind="ExternalInput")
    eye = nc.dram_tensor("eye", (P, P), _BF16, kind="ExternalInput")
    out = nc.dram_tensor("out", (TPC, H), _F32, kind="ExternalOutput")

    with tile.TileContext(nc) as tc:
        with (
            tc.tile_pool(name="consts", bufs=1) as consts,
            tc.tile_pool(name="w1p", bufs=1) as w1p,
            tc.tile_pool(name="w2p", bufs=1) as w2p,
            tc.tile_pool(name="hsup", bufs=2) as hsup,
            tc.tile_pool(name="xin", bufs=5) as xin,
            tc.tile_pool(name="rin", bufs=5) as rin,
            tc.tile_pool(name="lnp", bufs=4) as lnp,
            tc.tile_pool(name="lntp", bufs=8) as lntp,
            tc.tile_pool(name="intp", bufs=16) as intp,
            tc.tile_pool(name="resp", bufs=4) as resp,
            tc.tile_pool(name="stat", bufs=8) as stat,
            tc.tile_pool(name="ps_g1", bufs=3, space="PSUM") as ps_g1,
            tc.tile_pool(name="ps_g2", bufs=4, space="PSUM") as ps_g2,
            tc.tile_pool(name="ps_tr", bufs=1, space="PSUM") as ps_tr,
        ):
            eps_t = consts.tile([P, 1], _F32)
            nc.vector.memset(eps_t, EPS)

            # biases are ~0.02-std, so they ship as bf16: halves the 512KB
            # partition-broadcast DMAs (ab sits ahead of r0_t0 on gpsimd)
            ab_full = consts.tile([P, H], _BF16)
            ab_ap = ab[:]
            nc.gpsimd.dma_start(
                out=ab_full,
                in_=bass.AP(tensor=ab_ap.tensor, offset=ab_ap.offset,
                            ap=[[0, P]] + list(ab_ap.ap)),
            )
            ident = consts.tile([P, P], _BF16)
            nc.gpsimd.dma_start(out=ident, in_=eye[:, :])
            scratch = consts.tile([P, HCW], _BF16)
            nc.vector.memset(scratch, 0.25)

            ob_full = consts.tile([P, H], _BF16)
            b1_st = consts.tile([P, IC], _F32)
            nc.gpsimd.dma_start(out=b1_st, in_=b1[:].rearrange("(i p) -> p i", p=P))

            w1r = w1[:, :].rearrange("(ko p) i -> p ko i", p=P)
            w2r = w2[:, :].rearrange("(io p) h -> p io h", p=P)

            h_sups = [None] * N_SUPER
            a2_last = [None] * N_SUPER    # last lnT copy inst per supertile
            lnTs = [None] * N_SUPER       # fp8 per-k-pair tiles
            ln_ts = [None] * N_SUPER
            mvs = [None] * N_SUPER
            w1_ig = [None] * IG
            w2sb = [None] * (HC * 4)      # 8 resident chunk tiles

            x_tiles = {}
            r_tiles = {}

            def _last_inst():
                return nc.inst_map[next(reversed(nc.inst_map))]

            def emit_a1_dma(s):
                """x/r tile loads, alternating queues (2.1MB per queue)"""
                for t in range(ST):
                    g = s * ST + t
                    x_t = xin.tile([P, H], _F32, name=f"x_{g}", tag="x_t")
                    r_t = rin.tile([P, H], _F32, name=f"r_{g}", tag="r_t")
                    x_eng = nc.sync if t % 2 == 0 else nc.gpsimd
                    r_eng = nc.gpsimd if t % 2 == 0 else nc.sync
                    x_eng.dma_start(out=x_t, in_=x[g * P:(g + 1) * P, :])
                    r_eng.dma_start(out=r_t, in_=r[g * P:(g + 1) * P, :])
                    x_tiles[g] = x_t
                    r_tiles[g] = r_t

            def emit_a1_stats(s):
                """residual adds + LN stats (DVE only)"""
                h_sup = hsup.tile([P, ST, H], _F32, name=f"h_sup{s}", tag="h_sup")
                mv = stat.tile([P, ST, 2], _F32, name=f"mv_{s}", tag="mv")
                for t in range(ST):
                    g = s * ST + t
                    h_sl = h_sup[:, t, :]
                    nc.vector.tensor_add(h_sl, x_tiles[g], r_tiles[g])
                    nc.vector.tensor_add(h_sl, h_sl, ab_full)
                    stats = stat.tile([P, 2, 6], _F32, name=f"st_{g}", tag="stats")
                    for q in range(2):
                        nc.vector.bn_stats(out=stats[:, q, :],
                                           in_=h_sl[:, q * 512:(q + 1) * 512])
                    nc.vector.bn_aggr(out=mv[:, t, :], in_=stats)
                    if s == N_SUPER - 1:
                        # fold output bias into h here; its epilogue is the
                        # kernel tail, so halving the tail DVE work matters
                        nc.vector.tensor_add(h_sl, h_sl, ob_full)
                h_sups[s] = h_sup
                mvs[s] = mv

            def emit_a1_norm(s):
                """rstd (4 adjacent ACT Sqrt -> one table ctx) + bf16 ln"""
                mv = mvs[s]
                h_sup = h_sups[s]
                for t in range(ST):
                    nc.scalar.activation(
                        out=mv[:, t, 1:2], in_=mv[:, t, 1:2],
                        func=mybir.ActivationFunctionType.Sqrt,
                        bias=eps_t, scale=1.0)
                lns = []
                for t in range(ST):
                    nc.vector.reciprocal(out=mv[:, t, 1:2], in_=mv[:, t, 1:2])
                    g = s * ST + t
                    ln_t = lnp.tile([P, H], _BF16, name=f"ln_{g}", tag="ln_t")
                    nc.vector.tensor_scalar(
                        out=ln_t, in0=h_sup[:, t, :],
                        scalar1=mv[:, t, 0:1], scalar2=mv[:, t, 1:2],
                        op0=mybir.AluOpType.subtract, op1=mybir.AluOpType.mult,
                    )
                    lns.append(ln_t)
                ln_ts[s] = lns
                lnTs[s] = [lntp.tile([P, 2, ST_TOK], _FP8, name=f"lnT{s}_{j}",
                                     tag="lnT") for j in range(KP1)]

            def emit_a2_batch(s, k, on_dve=False):
                """4 PE transposes (one k across all 4 token tiles) into one
                psum bank + a single wide copy -> fp8 lnT pair half"""
                trp = ps_tr.tile([P, ST * P], _BF16, name=f"tr_{s}_{k}",
                                 tag="trp")
                for t in range(ST):
                    nc.tensor.transpose(trp[:, t * P:(t + 1) * P],
                                        ln_ts[s][t][:, k * P:(k + 1) * P],
                                        ident)
                dst = lnTs[s][k // 2][:, k % 2, :]
                nc.scalar.copy(out=dst, in_=trp)
                a2_last[s] = _last_inst()

            def emit_b(s):
                """GEMM1 (fp8 DoubleRow) + bias + exact GELU -> interT fp8"""
                pairs = [intp.tile([P, 2, ST_TOK], _FP8, name=f"int{s}_{j}",
                                   tag="interT") for j in range(IP2)]
                for i in range(IC):
                    pg1 = ps_g1.tile([P, ST_TOK], _F32, name=f"pg1_{s}_{i}",
                                     tag="pg1")
                    for kj in range(KP1):
                        nc.tensor.matmul(
                            pg1,
                            w1_ig[i // ICG][:, 2 * kj:2 * kj + 2,
                                            (i % ICG) * P:(i % ICG + 1) * P],
                            lnTs[s][kj][:, :, :],
                            start=(kj == 0), stop=(kj == KP1 - 1),
                            perf_mode=_DR)
                    nc.scalar.activation(out=pairs[i // 2][:, i % 2, :],
                                         in_=pg1,
                                         func=mybir.ActivationFunctionType.Gelu,
                                         bias=b1_st[:, i:i + 1],
                                         scale=1.0 / WSCALE)
                return pairs

            def emit_c(s, interT, next_s):
                """GEMM2 (fp8 DoubleRow, resident W2 chunks) in 4
                (hc, tq-pair) quarters + epilogue. s+1's lnT fp8 converts
                ride in the hc=1 quarters."""
                last = s == N_SUPER - 1
                a2_j = 0
                # steady state: next supertile's tokens/stats are ready by
                # mid-GEMM2, so its transposes ride one quarter earlier and
                # the lnT copies land before GEMM1(s+1) needs them; the
                # s=0->1 transition keeps them late (tokens-1 arrive late)
                batch_qs = (2, 3) if next_s == 1 else (1, 2)
                for hc in range(HC):
                    for th in range(2):
                        tqs = (2 * th, 2 * th + 1)
                        pg2s = [ps_g2.tile([P, HCW], _F32,
                                           name=f"pg2_{s}_{hc}_{tq}", tag="pg2")
                                for tq in tqs]
                        for ipi, ip in enumerate(range(0, IC, 2)):
                            wt = w2sb[hc * 4 + ip // W2B]
                            io = ip % W2B
                            for j, tq in enumerate(tqs):
                                nc.tensor.matmul(
                                    pg2s[j],
                                    interT[ip // 2][:, :, tq * P:(tq + 1) * P],
                                    wt[:, io:io + 2, :],
                                    start=(ip == 0), stop=(ip == IC - 2),
                                    perf_mode=_DR)
                            if (next_s is not None
                                    and (2 * hc + th) in batch_qs
                                    and ipi % 4 == 3 and a2_j < KO1):
                                emit_a2_batch(next_s, a2_j)
                                a2_j += 1
                        for j, tq in enumerate(tqs):
                            g = s * ST + tq
                            res_h = resp.tile([P, HCW], _F32,
                                              name=f"res_{s}_{hc}_{tq}",
                                              tag="res_h")
                            nc.scalar.activation(
                                out=res_h, in_=pg2s[j],
                                func=mybir.ActivationFunctionType.Copy,
                                scale=1.0 / WSCALE)
                            nc.vector.tensor_add(
                                res_h, res_h,
                                h_sups[s][:, tq, hc * HCW:(hc + 1) * HCW])
                            if not last:
                                nc.vector.tensor_add(
                                    res_h, res_h,
                                    ob_full[:, hc * HCW:(hc + 1) * HCW])
                            if last:
                                # kernel tail: fan the stores over 3 queues
                                st_eng = (nc.sync, nc.gpsimd,
                                          nc.scalar)[(2 * hc + th + j) % 3]
                            else:
                                st_eng = nc.gpsimd
                            st_eng.dma_start(
                                out=out[g * P:(g + 1) * P,
                                        hc * HCW:(hc + 1) * HCW],
                                in_=res_h)

            # ---- emission schedule (deadline-ordered DMA queues) ----
            emit_a1_dma(0)                  # x/r supertile 0, both queues
            # W1 groups by deadline: g0,g1 on sync and g2,g3 on gpsimd right
            # behind the supertile-0 tokens; g4..g7 after supertile-1's
            # tokens (those feed the sqrt/ln chain the ACT stream blocks on)
            kh = KO1 // 2

            def emit_w1_group(ig, eng):
                w1t = w1p.tile([P, KO1, ICG * P], _FP8, name=f"w1_{ig}",
                               tag=f"w1_{ig}")
                for q in range(2):
                    eng.dma_start(
                        out=w1t[:, q * kh:(q + 1) * kh, :],
                        in_=w1r[:, q * kh:(q + 1) * kh,
                                ig * ICG * P:(ig + 1) * ICG * P])
                w1_ig[ig] = w1t

            for ig in range(IG):
                emit_w1_group(ig, nc.sync if ig < 2 else nc.gpsimd)
            ob_ap = ob[:]
            nc.gpsimd.dma_start(
                out=ob_full,
                in_=bass.AP(tensor=ob_ap.tensor, offset=ob_ap.offset,
                            ap=[[0, P]] + list(ob_ap.ap)),
            )
            emit_a1_stats(0)
            emit_a1_norm(0)
            for k in range(KO1):            # supertile 0 fp8 lnT up front
                emit_a2_batch(0, k)
            for s in range(N_SUPER):
                if s + 1 < N_SUPER:
                    emit_a1_dma(s + 1)
                if s == 0:
                    # resident W2 chunks on sync behind supertile-1 tokens;
                    # hc=0 chunks first (needed from GEMM2(0) q1)
                    for hcq in range(HC):
                        for jb in range(4):
                            wt = w2p.tile([P, W2B, HCW], _FP8,
                                          name=f"w2_{hcq}_{jb}",
                                          tag=f"w2_{hcq}_{jb}")
                            nc.sync.dma_start(
                                out=wt,
                                in_=w2r[:, W2B * jb:W2B * (jb + 1),
                                        hcq * HCW:(hcq + 1) * HCW])
                            w2sb[hcq * 4 + jb] = wt
                interT = emit_b(s)
                if s + 1 < N_SUPER:
                    # ACT: sqrts land after the GELUs; DVE: A1 chain + ln;
                    # sync ring: the 4 XBAR transposes
                    emit_a1_stats(s + 1)
                    emit_a1_norm(s + 1)
                emit_c(s, interT, s + 1 if s + 1 < N_SUPER else None)

    nc.finalize()
    return nc


def kernel(input, residual, bias, attn_nw, attn_nb, inter_w, inter_b,
           output_w, output_b):
    global LAST_RESULT
    input = np.asarray(input, dtype=np.float32)
    residual = np.asarray(residual, dtype=np.float32)
    bias = np.asarray(bias, dtype=np.float32)
    attn_nw = np.asarray(attn_nw, dtype=np.float32)
    attn_nb = np.asarray(attn_nb, dtype=np.float32)
    inter_w = np.asarray(inter_w, dtype=np.float32)
    inter_b = np.asarray(inter_b, dtype=np.float32)
    output_w = np.asarray(output_w, dtype=np.float32)
    output_b = np.asarray(output_b, dtype=np.float32)

    x = np.ascontiguousarray(input.reshape(TOK, H))
    r = np.ascontiguousarray(residual.reshape(TOK, H))
    # fold LN affine params into GEMM1 weight/bias (exact algebra):
    #   (std*nw + nb) @ W1 + b1 == std @ (nw[:,None]*W1) + (nb @ W1 + b1)
    # then scale weights x128 into e4m3's normal range (TRN max +-240);
    # the GELU's scale=1/128 and the epilogue Copy scale undo it exactly.
    w1p = np.clip((attn_nw[:, None] * inter_w) * WSCALE, -240.0, 240.0)
    w1p = np.ascontiguousarray(w1p).astype(ml_dtypes.float8_e4m3)
    b1p = (attn_nb @ inter_w + inter_b).astype(np.float32)
    w2p = np.clip(output_w * WSCALE, -240.0, 240.0)
    w2p = np.ascontiguousarray(w2p).astype(ml_dtypes.float8_e4m3)
    eye = np.eye(P, dtype=ml_dtypes.bfloat16)

    nc = _build_nc()
    in_maps = []
    for c in range(N_CORES):
        in_maps.append({
            "x": np.ascontiguousarray(x[c * TPC:(c + 1) * TPC]),
            "r": np.ascontiguousarray(r[c * TPC:(c + 1) * TPC]),
            "w1": w1p, "b1": b1p, "w2": w2p,
            "ab": bias.astype(ml_dtypes.bfloat16),
            "ob": output_b.astype(ml_dtypes.bfloat16), "eye": eye,
        })
    res = run_bass_kernel_spmd(nc, in_maps, core_ids=list(range(N_CORES)),
                               trace=TRACE)
    LAST_RESULT = res
    out = np.concatenate([res.results[c]["out"] for c in range(N_CORES)], axis=0)
    return np.ascontiguousarray(out.reshape(B, S, H)).astype(np.float32)
